# revision 19
# baseline (speedup 1.0000x reference)
"""Augmented Neural ODE kernel for 8 TRN2 NeuronCores — fp8 SwInterleave variant.

Data-parallel over the batch dim (8 batches/core -> 512 tokens/core);
state kept feature-major [STATE=128 partitions, 512 tokens] in SBUF.
Layers 1-3 (contraction 1024) run as fp8e4m3 DoubleRowSwInterleave
matmuls: weights pre-interleaved host-side ([A127,B127,A126,B126,...]
per partition) so LDWEIGHTS reads contiguously — plain DoubleRow's
gather pays ~35ns/MM of extra PE-array time on top of the 216ns N=512
stream; SwInterleave removes it (measured 251 -> 220 ns/MM back-to-back).
Layer 0 runs in f32r straight off the carry (K=128 can't DoubleRow).
The Euler carry y' = y + dt*f rides layer 3's PSUM accumulation group
via an s3-scaled f32r identity matmul (s3 is the power-of-two fp8
scale of dt*W3, so the fold is lossless).

Note: the chip's power manager caps sustained density — schedules that
would finish below ~660us get every core-domain clock cut to 5/6 (PE
2.4->2.0GHz), which is a net loss. The m-outer emission order here
measures at full clock; denser hand schedules measured faster-per-clock
but throttled slower end-to-end.
"""

import sys

if "/opt/trn_rl_repo" not in sys.path:
    sys.path.insert(0, "/opt/trn_rl_repo")

import numpy as np

B, S, DIN, DAUG = 64, 64, 64, 64
STATE = DIN + DAUG          # 128
HID = 1024
T = 32
NCORES = 8
BSHARD = B // NCORES        # 8
NTOK = BSHARD * S           # 512 tokens per core
KC = HID // 128             # 8 chunks of the hidden dim
KP = KC // 2                # 4 chunk-pairs for DoubleRow

_cached = {}


def _build(scales):
    """scales = (s1, s2, s3) power-of-two per-matrix weight scales."""
    if scales in _cached:
        return _cached[scales]
    s1, s2, s3 = scales

    import concourse.tile as tile
    from concourse import bacc, mybir

    f32 = mybir.dt.float32
    f32r = mybir.dt.float32r
    fp8 = mybir.dt.float8e4
    SWI = mybir.MatmulPerfMode.DoubleRowSwInterleave
    Tanh = mybir.ActivationFunctionType.Tanh

    nc = bacc.Bacc("TRN2", target_bir_lowering=False, debug=False,
                   num_devices=NCORES)

    y0t_d = nc.dram_tensor("y0t", [DIN, NTOK], f32r, kind="ExternalInput").ap()
    laug_d = nc.dram_tensor("laug", [DIN, STATE], f32r, kind="ExternalInput").ap()
    w0t_d = nc.dram_tensor("w0t", [STATE, HID], f32r, kind="ExternalInput").ap()
    w1s_d = nc.dram_tensor("w1s", [128, KP, KC, 2, 128], fp8, kind="ExternalInput").ap()
    w2s_d = nc.dram_tensor("w2s", [128, KP, KC, 2, 128], fp8, kind="ExternalInput").ap()
    w3s_d = nc.dram_tensor("w3s", [128, KP, 2, STATE], fp8, kind="ExternalInput").ap()
    b0_d = nc.dram_tensor("b0", [128, KC], f32, kind="ExternalInput").ap()
    b1_d = nc.dram_tensor("b1", [128, KC], f32, kind="ExternalInput").ap()
    b2_d = nc.dram_tensor("b2", [128, KC], f32, kind="ExternalInput").ap()
    ybias_d = nc.dram_tensor("ybias", [STATE, T], f32, kind="ExternalInput").ap()
    out_d = nc.dram_tensor("out", [DIN, NTOK], f32r, kind="ExternalOutput").ap()

    with tile.TileContext(nc) as tc:
        with tc.tile_pool(name="wpool", bufs=1) as wpool, \
             tc.tile_pool(name="hpool", bufs=12) as hpool, \
             tc.tile_pool(name="ypool", bufs=2) as ypool, \
             tc.tile_pool(name="pspool", bufs=7, space="PSUM") as pspool, \
             tc.tile_pool(name="pypool", bufs=1, space="PSUM") as pypool:

            laug = wpool.tile([DIN, STATE], f32r)
            nc.scalar.dma_start(laug[:], laug_d[:])
            y0t = wpool.tile([DIN, NTOK], f32r)
            nc.sync.dma_start(y0t[0:DIN // 2], y0t_d[0:DIN // 2])
            nc.gpsimd.dma_start(y0t[DIN // 2:], y0t_d[DIN // 2:])
            ybias = wpool.tile([STATE, T], f32)
            nc.scalar.dma_start(ybias[:], ybias_d[:])
            w0t = wpool.tile([128, HID], f32r)
            nc.sync.dma_start(w0t[:, 0:HID // 2], w0t_d[:, 0:HID // 2])
            nc.gpsimd.dma_start(w0t[:, HID // 2:], w0t_d[:, HID // 2:])

            w1s = wpool.tile([128, KP, KC, 2, 128], fp8)
            nc.sync.dma_start(w1s[:, 0], w1s_d[:, 0])
            nc.gpsimd.dma_start(w1s[:, 1], w1s_d[:, 1])
            nc.scalar.dma_start(w1s[:, 2], w1s_d[:, 2])
            nc.gpsimd.dma_start(w1s[:, 3], w1s_d[:, 3])
            w2s = wpool.tile([128, KP, KC, 2, 128], fp8)
            nc.sync.dma_start(w2s[:, 0], w2s_d[:, 0])
            nc.gpsimd.dma_start(w2s[:, 1], w2s_d[:, 1])
            nc.scalar.dma_start(w2s[:, 2], w2s_d[:, 2])
            nc.sync.dma_start(w2s[:, 3], w2s_d[:, 3])
            w3s = wpool.tile([128, KP, 2, STATE], fp8)
            nc.gpsimd.dma_start(w3s[:], w3s_d[:])
            b0 = wpool.tile([128, KC], f32)
            nc.sync.dma_start(b0[:], b0_d[:])
            b1 = wpool.tile([128, KC], f32)
            nc.sync.dma_start(b1[:], b1_d[:])
            b2 = wpool.tile([128, KC], f32)
            nc.sync.dma_start(b2[:], b2_d[:])

            # augment into the persistent carry bank: ps_y = s3*[y0; W_aug y0]
            # (laug is pre-scaled by s3 host-side; biases telescope into the
            # per-step copy via the ybias table: col n = baug + n*dt*b3)
            ps_y = pypool.tile([128, NTOK], f32, tag="ps_y")
            nc.tensor.matmul(ps_y[:], lhsT=laug[:], rhs=y0t[:],
                             start=True, stop=True)
            y = ypool.tile([128, NTOK], f32r, tag="y")
            nc.vector.tensor_scalar(y[:], ps_y[:], 1.0 / s3, ybias[:, 0:1],
                                    mybir.AluOpType.mult,
                                    mybir.AluOpType.add)

            for _step in range(T - 1):
                # layer 0: f32r straight off the carry y
                h0 = [hpool.tile([128, 2, NTOK], fp8, tag="h", name=f"h0_{_step}_{i}")
                      for i in range(KP)]
                for m in range(KC):
                    ps = pspool.tile([128, NTOK], f32, tag="ps")
                    nc.tensor.matmul(ps[:], lhsT=w0t[:, m * 128:(m + 1) * 128],
                                     rhs=y[:], start=True, stop=True)
                    nc.scalar.activation(h0[m // 2][:, m % 2, :], ps[:], Tanh,
                                         bias=b0[:, m:m + 1])
                # layer 1: fp8 SwInterleave, K=256 per matmul
                h1 = [hpool.tile([128, 2, NTOK], fp8, tag="h", name=f"h1_{_step}_{i}")
                      for i in range(KP)]
                for m in range(KC):
                    ps = pspool.tile([128, NTOK], f32, tag="ps")
                    for k in range(KP):
                        nc.tensor.matmul(ps[:], lhsT=w1s[:, k, m],
                                         rhs=h0[k][:],
                                         start=(k == 0), stop=(k == KP - 1),
                                         perf_mode=SWI)
                    nc.scalar.activation(h1[m // 2][:, m % 2, :], ps[:], Tanh,
                                         bias=b1[:, m:m + 1], scale=1.0 / s1)
                # layer 2 with layer 3's matmuls interleaved as their h2
                # pairs become ready; the Euler carry rides the same PSUM
                # group via the s3-scaled f32r identity matmul
                h2 = [hpool.tile([128, 2, NTOK], fp8, tag="h", name=f"h2_{_step}_{i}")
                      for i in range(KP)]
                for m in range(KC):
                    ps = pspool.tile([128, NTOK], f32, tag="ps")
                    for k in range(KP):
                        nc.tensor.matmul(ps[:], lhsT=w2s[:, k, m],
                                         rhs=h1[k][:],
                                         start=(k == 0), stop=(k == KP - 1),
                                         perf_mode=SWI)
                    nc.scalar.activation(h2[m // 2][:, m % 2, :], ps[:], Tanh,
                                         bias=b2[:, m:m + 1], scale=1.0 / s2)
                    if m == 3 or m == 5 or m == 7:
                        k = (m - 3) // 2
                        nc.tensor.matmul(ps_y[:], lhsT=w3s[:, k],
                                         rhs=h2[k][:],
                                         start=False, stop=False,
                                         perf_mode=SWI,
                                         skip_group_check=True)
                nc.tensor.matmul(ps_y[:], lhsT=w3s[:, 3], rhs=h2[3][:],
                                 start=False, stop=True, perf_mode=SWI,
                                 skip_group_check=True)
                # carry on the vector engine; scalar stays free for tanhs
                y = ypool.tile([128, NTOK], f32r, tag="y")
                nc.vector.tensor_scalar(y[:], ps_y[:], 1.0 / s3,
                                        ybias[:, _step + 1:_step + 2],
                                        mybir.AluOpType.mult,
                                        mybir.AluOpType.add)

            nc.sync.dma_start(out_d[:], y[0:DIN, :])

    nc.compile()
    _cached[scales] = nc
    return nc


def _pow2_scale(W, target=224.0):
    import math
    return 2.0 ** math.floor(math.log2(target / float(np.abs(W).max())))


def _swi_pairs(Wt):
    """Wt: [K_in, M_out] (lhsT orientation) with K_in = 256*kp.
    Returns [128, kp, M_out//128, 2, 128] in SwInterleave layout:
    per partition the 256 weights of a (k, m) chunk are
    [A_{127}, B_{127}, ..., A_0, B_0] with A/B = K-subchunks 2k/2k+1
    and columns reversed."""
    K_in, M_out = Wt.shape
    kp = K_in // 256
    mc = M_out // 128
    out = np.empty((128, kp, mc, 2, 128), np.float32)
    for k in range(kp):
        lo = Wt[(2 * k) * 128:(2 * k + 1) * 128]
        hi = Wt[(2 * k + 1) * 128:(2 * k + 2) * 128]
        for m in range(mc):
            ms = slice(m * 128, (m + 1) * 128)
            pair = np.stack([lo[:, ms], hi[:, ms]], axis=1)  # [128, 2, 128]
            tmp = pair[:, :, ::-1].transpose(0, 2, 1)        # [128, 128, 2]
            out[:, k, m] = tmp.reshape(128, 2, 128)
    return out


def _make_in_maps(y0, t, W_aug, b_aug, W0, b0, W1, b1, W2, b2, W3, b3):
    import ml_dtypes
    f = np.float32
    f8 = ml_dtypes.float8_e4m3
    dt = float(np.asarray(t, dtype=f)[1] - np.asarray(t, dtype=f)[0])
    W1, W2 = np.asarray(W1, f), np.asarray(W2, f)
    W3dt = dt * np.asarray(W3, f)
    s1, s2, s3 = _pow2_scale(W1), _pow2_scale(W2), _pow2_scale(W3dt)

    laug = s3 * np.concatenate([np.eye(DIN, dtype=f),
                                np.asarray(W_aug, f).T], axis=1)
    w0t = np.ascontiguousarray(np.asarray(W0, f).T)
    w1s = np.ascontiguousarray(_swi_pairs((W1 * s1).T)).astype(f8)
    w2s = np.ascontiguousarray(_swi_pairs((W2 * s2).T)).astype(f8)
    w3s = np.ascontiguousarray(
        _swi_pairs((W3dt * s3).T)[:, :, 0]).astype(f8)  # [128, KP, 2, 128]
    b0r = np.ascontiguousarray(np.asarray(b0, f).reshape(KC, 128).T)
    b1r = np.ascontiguousarray(np.asarray(b1, f).reshape(KC, 128).T)
    b2r = np.ascontiguousarray(np.asarray(b2, f).reshape(KC, 128).T)
    baug_full = np.concatenate([np.zeros(DIN, f), np.asarray(b_aug, f)])
    b3dt = dt * np.asarray(b3, f)
    ybias = np.ascontiguousarray(
        baug_full[:, None] + np.arange(T, dtype=f)[None, :] * b3dt[:, None])

    shared = dict(laug=laug, w0t=w0t, w1s=w1s, w2s=w2s, w3s=w3s,
                  b0=b0r, b1=b1r, b2=b2r, ybias=ybias)
    in_maps = []
    for c in range(NCORES):
        y0c = np.ascontiguousarray(
            np.asarray(y0, f)[c * BSHARD:(c + 1) * BSHARD]
            .reshape(NTOK, DIN).T)
        in_maps.append(dict(y0t=y0c, **shared))
    return in_maps, (s1, s2, s3)


def _run(inputs, trace=False, **trace_kwargs):
    from concourse.bass_utils import run_bass_kernel_spmd

    in_maps, scales = _make_in_maps(**inputs)
    nc = _build(scales)
    res = run_bass_kernel_spmd(nc, in_maps, core_ids=list(range(NCORES)),
                               trace=trace, **trace_kwargs)
    outs = [res.results[c]["out"] for c in range(NCORES)]
    full = np.concatenate(
        [o.T.reshape(BSHARD, S, DIN) for o in outs], axis=0)
    return np.ascontiguousarray(full, dtype=np.float32), res


def kernel(**inputs):
    out, _ = _run(inputs, trace=False)
    return out


# revision 21
# speedup vs baseline: 1.0007x; 1.0007x over previous
"""Augmented Neural ODE kernel for 8 TRN2 NeuronCores — fp8 SwInterleave variant.

Data-parallel over the batch dim (8 batches/core -> 512 tokens/core);
state kept feature-major [STATE=128 partitions, 512 tokens] in SBUF.
Layers 1-3 (contraction 1024) run as fp8e4m3 DoubleRowSwInterleave
matmuls: weights pre-interleaved host-side ([A127,B127,A126,B126,...]
per partition) so LDWEIGHTS reads contiguously — plain DoubleRow's
gather pays ~35ns/MM of extra PE-array time on top of the 216ns N=512
stream; SwInterleave removes it (measured 251 -> 220 ns/MM back-to-back).
Layer 0 runs in f32r straight off the carry (K=128 can't DoubleRow).
The Euler carry lives in one PSUM bank for the whole kernel: layer 3's
matmuls accumulate s3*(y_aug + sum dt*W3 h2) across all 31 steps (s3 is
the power-of-two fp8 scale of dt*W3, so the factor is exact), and each
step the vector engine materializes y = ps_y/s3 + (baug + n*dt*b3) into
SBUF from a precomputed per-step bias table. This removes the per-step
identity matmul an earlier variant used and keeps y at full f32 PSUM
precision between steps (no f32r round-trip).

Note: the chip's power manager caps sustained density — schedules that
would finish below ~660us get every core-domain clock cut to 5/6 (PE
2.4->2.0GHz), which is a net loss. The m-outer emission order here
measures at full clock; denser hand schedules measured faster-per-clock
but throttled slower end-to-end.
"""

import sys

if "/opt/trn_rl_repo" not in sys.path:
    sys.path.insert(0, "/opt/trn_rl_repo")

import numpy as np

B, S, DIN, DAUG = 64, 64, 64, 64
STATE = DIN + DAUG          # 128
HID = 1024
T = 32
NCORES = 8
BSHARD = B // NCORES        # 8
NTOK = BSHARD * S           # 512 tokens per core
KC = HID // 128             # 8 chunks of the hidden dim
KP = KC // 2                # 4 chunk-pairs for DoubleRow

_cached = {}


def _build(scales):
    """scales = (s1, s2, s3) power-of-two per-matrix weight scales."""
    if scales in _cached:
        return _cached[scales]
    s1, s2, s3 = scales

    import concourse.tile as tile
    from concourse import bacc, mybir

    f32 = mybir.dt.float32
    f32r = mybir.dt.float32r
    fp8 = mybir.dt.float8e4
    SWI = mybir.MatmulPerfMode.DoubleRowSwInterleave
    Tanh = mybir.ActivationFunctionType.Tanh

    nc = bacc.Bacc("TRN2", target_bir_lowering=False, debug=False,
                   num_devices=NCORES)

    y0t_d = nc.dram_tensor("y0t", [DIN, NTOK], f32r, kind="ExternalInput").ap()
    laug_d = nc.dram_tensor("laug", [DIN, STATE], f32r, kind="ExternalInput").ap()
    w0t_d = nc.dram_tensor("w0t", [STATE, HID], f32r, kind="ExternalInput").ap()
    w1s_d = nc.dram_tensor("w1s", [128, KP, KC, 2, 128], fp8, kind="ExternalInput").ap()
    w2s_d = nc.dram_tensor("w2s", [128, KP, KC, 2, 128], fp8, kind="ExternalInput").ap()
    w3s_d = nc.dram_tensor("w3s", [128, KP, 2, STATE], fp8, kind="ExternalInput").ap()
    b0_d = nc.dram_tensor("b0", [128, KC], f32, kind="ExternalInput").ap()
    b1_d = nc.dram_tensor("b1", [128, KC], f32, kind="ExternalInput").ap()
    b2_d = nc.dram_tensor("b2", [128, KC], f32, kind="ExternalInput").ap()
    ybias_d = nc.dram_tensor("ybias", [STATE, T], f32, kind="ExternalInput").ap()
    out_d = nc.dram_tensor("out", [DIN, NTOK], f32r, kind="ExternalOutput").ap()

    with tile.TileContext(nc) as tc:
        with tc.tile_pool(name="wpool", bufs=1) as wpool, \
             tc.tile_pool(name="hpool", bufs=12) as hpool, \
             tc.tile_pool(name="ypool", bufs=2) as ypool, \
             tc.tile_pool(name="pspool", bufs=7, space="PSUM") as pspool, \
             tc.tile_pool(name="pypool", bufs=1, space="PSUM") as pypool:

            laug = wpool.tile([DIN, STATE], f32r)
            nc.scalar.dma_start(laug[:], laug_d[:])
            y0t = wpool.tile([DIN, NTOK], f32r)
            nc.sync.dma_start(y0t[0:DIN // 2], y0t_d[0:DIN // 2])
            nc.gpsimd.dma_start(y0t[DIN // 2:], y0t_d[DIN // 2:])
            ybias = wpool.tile([STATE, T], f32)
            nc.scalar.dma_start(ybias[:], ybias_d[:])
            w0t = wpool.tile([128, HID], f32r)
            nc.sync.dma_start(w0t[:, 0:HID // 2], w0t_d[:, 0:HID // 2])
            nc.gpsimd.dma_start(w0t[:, HID // 2:], w0t_d[:, HID // 2:])

            w1s = wpool.tile([128, KP, KC, 2, 128], fp8)
            nc.sync.dma_start(w1s[:, 0], w1s_d[:, 0])
            nc.gpsimd.dma_start(w1s[:, 1], w1s_d[:, 1])
            nc.scalar.dma_start(w1s[:, 2], w1s_d[:, 2])
            nc.gpsimd.dma_start(w1s[:, 3], w1s_d[:, 3])
            w2s = wpool.tile([128, KP, KC, 2, 128], fp8)
            nc.sync.dma_start(w2s[:, 0], w2s_d[:, 0])
            nc.gpsimd.dma_start(w2s[:, 1], w2s_d[:, 1])
            nc.scalar.dma_start(w2s[:, 2], w2s_d[:, 2])
            nc.sync.dma_start(w2s[:, 3], w2s_d[:, 3])
            w3s = wpool.tile([128, KP, 2, STATE], fp8)
            nc.gpsimd.dma_start(w3s[:], w3s_d[:])
            b0 = wpool.tile([128, KC], f32)
            nc.sync.dma_start(b0[:], b0_d[:])
            b1 = wpool.tile([128, KC], f32)
            nc.sync.dma_start(b1[:], b1_d[:])
            b2 = wpool.tile([128, KC], f32)
            nc.sync.dma_start(b2[:], b2_d[:])

            # augment into the persistent carry bank: ps_y = s3*[y0; W_aug y0]
            # (laug is pre-scaled by s3 host-side; biases telescope into the
            # per-step copy via the ybias table: col n = baug + n*dt*b3)
            ps_y = pypool.tile([128, NTOK], f32, tag="ps_y")
            nc.tensor.matmul(ps_y[:], lhsT=laug[:], rhs=y0t[:],
                             start=True, stop=True)
            y = ypool.tile([128, NTOK], f32r, tag="y")
            nc.vector.tensor_scalar(y[:], ps_y[:], 1.0 / s3, ybias[:, 0:1],
                                    mybir.AluOpType.mult,
                                    mybir.AluOpType.add)

            for _step in range(T - 1):
                # layer 0: f32r straight off the carry y
                h0 = [hpool.tile([128, 2, NTOK], fp8, tag="h", name=f"h0_{_step}_{i}")
                      for i in range(KP)]
                hf = NTOK // 2
                for m in range(KC):
                    ps = pspool.tile([128, NTOK], f32, tag="ps")
                    nc.tensor.matmul(ps[:, 0:hf],
                                     lhsT=w0t[:, m * 128:(m + 1) * 128],
                                     rhs=y[:, 0:hf], start=True, stop=False)
                    nc.tensor.matmul(ps[:, hf:],
                                     lhsT=w0t[:, m * 128:(m + 1) * 128],
                                     rhs=y[:, hf:], start=False, stop=True)
                    nc.scalar.activation(h0[m // 2][:, m % 2, :], ps[:], Tanh,
                                         bias=b0[:, m:m + 1])
                # layer 1: fp8 SwInterleave, K=256 per matmul
                h1 = [hpool.tile([128, 2, NTOK], fp8, tag="h", name=f"h1_{_step}_{i}")
                      for i in range(KP)]
                for m in range(KC):
                    ps = pspool.tile([128, NTOK], f32, tag="ps")
                    for k in range(KP):
                        nc.tensor.matmul(ps[:], lhsT=w1s[:, k, m],
                                         rhs=h0[k][:],
                                         start=(k == 0), stop=(k == KP - 1),
                                         perf_mode=SWI)
                    nc.scalar.activation(h1[m // 2][:, m % 2, :], ps[:], Tanh,
                                         bias=b1[:, m:m + 1], scale=1.0 / s1)
                # layer 2 with layer 3's matmuls interleaved as their h2
                # pairs become ready; the Euler carry rides the same PSUM
                # group via the s3-scaled f32r identity matmul
                h2 = [hpool.tile([128, 2, NTOK], fp8, tag="h", name=f"h2_{_step}_{i}")
                      for i in range(KP)]
                for m in range(KC):
                    ps = pspool.tile([128, NTOK], f32, tag="ps")
                    for k in range(KP):
                        nc.tensor.matmul(ps[:], lhsT=w2s[:, k, m],
                                         rhs=h1[k][:],
                                         start=(k == 0), stop=(k == KP - 1),
                                         perf_mode=SWI)
                    nc.scalar.activation(h2[m // 2][:, m % 2, :], ps[:], Tanh,
                                         bias=b2[:, m:m + 1], scale=1.0 / s2)
                    if m == 3 or m == 5 or m == 7:
                        k = (m - 3) // 2
                        nc.tensor.matmul(ps_y[:], lhsT=w3s[:, k],
                                         rhs=h2[k][:],
                                         start=False, stop=False,
                                         perf_mode=SWI,
                                         skip_group_check=True)
                nc.tensor.matmul(ps_y[:], lhsT=w3s[:, 3], rhs=h2[3][:],
                                 start=False, stop=True, perf_mode=SWI,
                                 skip_group_check=True)
                # carry on the vector engine in two token halves; layer 0's
                # first-half matmuls overlap the second half's copy
                y = ypool.tile([128, NTOK], f32r, tag="y")
                hf = NTOK // 2
                nc.vector.tensor_scalar(y[:, 0:hf], ps_y[:, 0:hf], 1.0 / s3,
                                        ybias[:, _step + 1:_step + 2],
                                        mybir.AluOpType.mult,
                                        mybir.AluOpType.add)
                nc.vector.tensor_scalar(y[:, hf:], ps_y[:, hf:], 1.0 / s3,
                                        ybias[:, _step + 1:_step + 2],
                                        mybir.AluOpType.mult,
                                        mybir.AluOpType.add)

            nc.sync.dma_start(out_d[:], y[0:DIN, :])

    nc.compile()
    _cached[scales] = nc
    return nc


def _pow2_scale(W, target=224.0):
    import math
    return 2.0 ** math.floor(math.log2(target / float(np.abs(W).max())))


def _swi_pairs(Wt):
    """Wt: [K_in, M_out] (lhsT orientation) with K_in = 256*kp.
    Returns [128, kp, M_out//128, 2, 128] in SwInterleave layout:
    per partition the 256 weights of a (k, m) chunk are
    [A_{127}, B_{127}, ..., A_0, B_0] with A/B = K-subchunks 2k/2k+1
    and columns reversed."""
    K_in, M_out = Wt.shape
    kp = K_in // 256
    mc = M_out // 128
    out = np.empty((128, kp, mc, 2, 128), np.float32)
    for k in range(kp):
        lo = Wt[(2 * k) * 128:(2 * k + 1) * 128]
        hi = Wt[(2 * k + 1) * 128:(2 * k + 2) * 128]
        for m in range(mc):
            ms = slice(m * 128, (m + 1) * 128)
            pair = np.stack([lo[:, ms], hi[:, ms]], axis=1)  # [128, 2, 128]
            tmp = pair[:, :, ::-1].transpose(0, 2, 1)        # [128, 128, 2]
            out[:, k, m] = tmp.reshape(128, 2, 128)
    return out


def _make_in_maps(y0, t, W_aug, b_aug, W0, b0, W1, b1, W2, b2, W3, b3):
    import ml_dtypes
    f = np.float32
    f8 = ml_dtypes.float8_e4m3
    dt = float(np.asarray(t, dtype=f)[1] - np.asarray(t, dtype=f)[0])
    W1, W2 = np.asarray(W1, f), np.asarray(W2, f)
    W3dt = dt * np.asarray(W3, f)
    s1, s2, s3 = _pow2_scale(W1), _pow2_scale(W2), _pow2_scale(W3dt)

    laug = s3 * np.concatenate([np.eye(DIN, dtype=f),
                                np.asarray(W_aug, f).T], axis=1)
    w0t = np.ascontiguousarray(np.asarray(W0, f).T)
    w1s = np.ascontiguousarray(_swi_pairs((W1 * s1).T)).astype(f8)
    w2s = np.ascontiguousarray(_swi_pairs((W2 * s2).T)).astype(f8)
    w3s = np.ascontiguousarray(
        _swi_pairs((W3dt * s3).T)[:, :, 0]).astype(f8)  # [128, KP, 2, 128]
    b0r = np.ascontiguousarray(np.asarray(b0, f).reshape(KC, 128).T)
    b1r = np.ascontiguousarray(np.asarray(b1, f).reshape(KC, 128).T)
    b2r = np.ascontiguousarray(np.asarray(b2, f).reshape(KC, 128).T)
    baug_full = np.concatenate([np.zeros(DIN, f), np.asarray(b_aug, f)])
    b3dt = dt * np.asarray(b3, f)
    ybias = np.ascontiguousarray(
        baug_full[:, None] + np.arange(T, dtype=f)[None, :] * b3dt[:, None])

    shared = dict(laug=laug, w0t=w0t, w1s=w1s, w2s=w2s, w3s=w3s,
                  b0=b0r, b1=b1r, b2=b2r, ybias=ybias)
    in_maps = []
    for c in range(NCORES):
        y0c = np.ascontiguousarray(
            np.asarray(y0, f)[c * BSHARD:(c + 1) * BSHARD]
            .reshape(NTOK, DIN).T)
        in_maps.append(dict(y0t=y0c, **shared))
    return in_maps, (s1, s2, s3)


def _run(inputs, trace=False, **trace_kwargs):
    from concourse.bass_utils import run_bass_kernel_spmd

    in_maps, scales = _make_in_maps(**inputs)
    nc = _build(scales)
    res = run_bass_kernel_spmd(nc, in_maps, core_ids=list(range(NCORES)),
                               trace=trace, **trace_kwargs)
    outs = [res.results[c]["out"] for c in range(NCORES)]
    full = np.concatenate(
        [o.T.reshape(BSHARD, S, DIN) for o in outs], axis=0)
    return np.ascontiguousarray(full, dtype=np.float32), res


def kernel(**inputs):
    out, _ = _run(inputs, trace=False)
    return out
